# revision 1
# baseline (speedup 1.0000x reference)
"""Trainium2 Bass kernel for MatrixGraphConvolution.

out = D^-1 A (x @ W.T) + x @ B.T,  A[dst,src]=1 (set semantics),
deg counts duplicate edges, N=16384, E=524288, F=128.

Strategy (8 NeuronCores, row-sharded by dst):
  * Host builds the 0/1 adjacency-transpose shards A^T[src, dst_rel] as
    fp8e4m3 (values 0/1 are exact) - 32MB per core, the dominant HBM read.
  * Each core redundantly computes xW = x @ W.T on device (fp16 in, fp32
    PSUM accumulate, fp16 stationary out).
  * agg^T[f, dst] = xW.T @ A^T accumulated over 128 src-chunks in PSUM;
    moving operand is the fp8 A^T streamed from HBM.
  * Residual x @ B.T is folded into the same PSUM accumulation with x
    pre-scaled by deg on the host, so the final per-dst 1/deg scale
    applies to both terms: out = invdeg * (A xW + deg * (x B^T)).
  * Eviction: one fp32 multiply by the broadcast invdeg, DMA out^T.
"""

import sys

sys.path.insert(0, "/opt/trn_rl_repo")

import numpy as np
import ml_dtypes

import concourse.bass as bass
import concourse.tile as tile
import concourse.mybir as mybir
from concourse import bacc
from concourse.bass import ts
from concourse.bass_utils import run_bass_kernel_spmd

N, E, F = 16384, 524288, 128
NCORES = 8
SH = N // NCORES          # 2048 dst rows per core
NCH = N // 128            # 128 src chunks
GRP = 4                   # src chunks per DMA group (1MB per DMA)
NGRP = NCH // GRP
NB = SH // 512            # moving-dim blocks (PSUM banks) per chunk

FP16 = mybir.dt.float16
FP8 = mybir.dt.float8e4
FP32 = mybir.dt.float32

_NC = None


def _build():
    global _NC
    if _NC is not None:
        return _NC
    nc = bacc.Bacc(None, target_bir_lowering=False)
    at = nc.dram_tensor("at", [N, SH], FP8, kind="ExternalInput")
    xt = nc.dram_tensor("xt", [F, N], FP16, kind="ExternalInput")
    xtc = nc.dram_tensor("xtc", [F, SH], FP16, kind="ExternalInput")
    wt = nc.dram_tensor("wt", [F, F], FP16, kind="ExternalInput")
    bt = nc.dram_tensor("bt", [F, F], FP16, kind="ExternalInput")
    idb = nc.dram_tensor("idb", [F, SH], FP32, kind="ExternalInput")
    outT = nc.dram_tensor("outT", [F, SH], FP32, kind="ExternalOutput")

    with tile.TileContext(nc) as tc:
        with (
            tc.tile_pool(name="const", bufs=1) as constp,
            tc.tile_pool(name="xw", bufs=1) as xwp,
            tc.tile_pool(name="apool", bufs=3) as apool,
            tc.tile_pool(name="outp", bufs=1) as outp,
            tc.tile_pool(name="psA", bufs=1, space=bass.MemorySpace.PSUM) as psA,
            tc.tile_pool(name="psW", bufs=2, space=bass.MemorySpace.PSUM) as psW,
        ):
            xt_sb = constp.tile([F, N], FP16, tag="xt")
            nc.sync.dma_start(xt_sb[:], xt[:])
            xtc_sb = constp.tile([F, SH], FP16, tag="xtc")
            nc.sync.dma_start(xtc_sb[:], xtc[:])
            wt_sb = constp.tile([F, F], FP16, tag="wt")
            nc.sync.dma_start(wt_sb[:], wt[:])
            bt_sb = constp.tile([F, F], FP16, tag="bt")
            nc.sync.dma_start(bt_sb[:], bt[:])
            idb_sb = constp.tile([F, SH], FP32, tag="idb")
            nc.sync.dma_start(idb_sb[:], idb[:])

            # xW = x @ W.T; tile t holds rows [t*128,(t+1)*128) on partitions.
            xw_sb = xwp.tile([128, N], FP16, tag="xw")
            for t in range(0, NCH, 4):
                pw = psW.tile([128, 512], FP32, tag="pw")
                for q in range(4):
                    nc.tensor.matmul(
                        pw[:, ts(q, 128)],
                        xt_sb[:, ts(t + q, 128)],
                        wt_sb[:],
                        start=True,
                        stop=True,
                    )
                nc.vector.tensor_copy(xw_sb[:, ts(t // 4, 512)], pw[:])

            ps = psA.tile([128, SH], FP32, tag="agg")
            # residual first: ps[f, n] += sum_c B^T[c, f] * (deg*x)^T[c, n]
            for b in range(NB):
                nc.tensor.matmul(
                    ps[:, ts(b, 512)],
                    bt_sb[:],
                    xtc_sb[:, ts(b, 512)],
                    start=True,
                    stop=False,
                )
            # aggregation: ps[f, dst] += sum_src xW[src, f] * A^T[src, dst]
            for g in range(NGRP):
                a_t = apool.tile([128, GRP, SH], FP8, tag="a")
                in_ap = bass.AP(
                    at,
                    g * GRP * 128 * SH,
                    [[SH, 128], [128 * SH, GRP], [1, SH]],
                )
                nc.sync.dma_start(a_t[:], in_ap)
                for j in range(GRP):
                    c = g * GRP + j
                    last = c == NCH - 1
                    for b in range(NB):
                        nc.tensor.matmul(
                            ps[:, ts(b, 512)],
                            xw_sb[:, ts(c, 128)],
                            a_t[:, j, ts(b, 512)],
                            start=False,
                            stop=last,
                        )
            out_sb = outp.tile([128, SH], FP32, tag="out")
            nc.vector.tensor_mul(out_sb[:], ps[:], idb_sb[:])
            nc.sync.dma_start(outT[:], out_sb[:])

    nc.compile()
    _NC = nc
    return nc


def _prep_inputs(x, edge_index, W, B):
    src = np.asarray(edge_index[0])
    dst = np.asarray(edge_index[1])
    x = np.asarray(x, dtype=np.float32)
    W = np.asarray(W, dtype=np.float32)
    B = np.asarray(B, dtype=np.float32)

    deg = np.bincount(dst, minlength=N).astype(np.float32)
    dtil = np.where(deg == 0, np.float32(1.0), deg)
    invdeg = (np.float32(1.0) / dtil).astype(np.float32)

    # A^T shards: at_all[k, src, dst & 2047] = 1.0 (fp8 bit pattern 0x38),
    # duplicate edges naturally collapse (set semantics).
    at_all = np.zeros((NCORES, N, SH), dtype=np.uint8)
    at_all[dst >> 11, src, dst & (SH - 1)] = 0x38
    at_all = at_all.view(ml_dtypes.float8_e4m3)

    xt_np = np.ascontiguousarray(x.T).astype(np.float16)
    xtil = (dtil[:, None] * x).astype(np.float16)
    wt_np = np.ascontiguousarray(W.T).astype(np.float16)
    bt_np = np.ascontiguousarray(B.T).astype(np.float16)

    in_maps = []
    for k in range(NCORES):
        sl = slice(k * SH, (k + 1) * SH)
        in_maps.append(
            {
                "at": at_all[k],
                "xt": xt_np,
                "xtc": np.ascontiguousarray(xtil[sl].T),
                "wt": wt_np,
                "bt": bt_np,
                "idb": np.ascontiguousarray(
                    np.broadcast_to(invdeg[sl][None, :], (F, SH))
                ).astype(np.float32),
            }
        )
    return in_maps


def kernel(x, edge_index, W, B):
    nc = _build()
    in_maps = _prep_inputs(x, edge_index, W, B)
    res = run_bass_kernel_spmd(nc, in_maps, core_ids=list(range(NCORES)))
    out = np.empty((N, F), dtype=np.float32)
    for k in range(NCORES):
        out[k * SH : (k + 1) * SH, :] = res.results[k]["outT"].T
    return out


# revision 15
# speedup vs baseline: 26930.4534x; 26930.4534x over previous
"""Trainium2 Bass kernel for MatrixGraphConvolution.

out = D^-1 A (x @ W.T) + x @ B.T,  A[dst,src]=1 (set semantics),
deg counts duplicate edges, N=16384, E=524288, F=128.

Strategy (8 NeuronCores, row-sharded by dst):
  * Host builds the 0/1 adjacency-transpose shards A^T[src, dst_rel] as
    fp8e4m3 (values 0/1 are exact), laid out in the exact SBUF tile
    order so every A DMA is fully contiguous - 32MB per core, the
    dominant HBM read (streamed on the Sync HWDGE ring; everything else
    rides the Scalar ring).
  * Associativity: A@(x@W.T) = ((x^T @ A^T)^T @ W.T). The main loop
    accumulates Y[f_in, dst] = x^T @ A^T over 128 src-chunks in PSUM
    with raw x chunks as the stationary operand (no xW precompute
    phase) and the fp8 A^T as the moving operand (mixed fp16 x fp8
    matmul - verified exact on HW). W is applied once at the end:
    agg^T = W @ Y (4 matmuls).
  * Residual x @ B.T accumulates in the second PSUM region with x
    pre-scaled by deg on the host, so the final per-dst 1/deg scale
    applies to both terms: out = invdeg * (A xW + deg * (x B^T)).
  * invdeg row is partition-broadcast on-device via a stride-0 DMA,
    then one fp32 multiply per PSUM bank at eviction, DMA out^T.
"""

import sys

sys.path.insert(0, "/opt/trn_rl_repo")

import numpy as np
import ml_dtypes

import concourse.bass as bass
import concourse.tile as tile
import concourse.mybir as mybir
from concourse import bacc
from concourse.bass import ts
from concourse.bass_utils import run_bass_kernel_spmd

N, E, F = 16384, 524288, 128
NCORES = 8
SH = N // NCORES          # 2048 dst rows per core
SHB = 11                  # log2(SH)
NCH = N // 128            # 128 src chunks
GRP = 4                   # src chunks per DMA group (1MB per DMA)
NGRP = NCH // GRP
NB = SH // 512            # moving-dim blocks (PSUM banks) per chunk
XP = 8                    # x pieces (16 chunks each)
CPP = NCH // XP           # chunks per x piece

FP16 = mybir.dt.float16
FP8 = mybir.dt.float8e4
FP32 = mybir.dt.float32

_NC = None


def _build():
    global _NC
    if _NC is not None:
        return _NC
    nc = bacc.Bacc(None, target_bir_lowering=False)
    # at is pre-tiled on the host: at[g][p][j*SH + d] = A^T[(g*GRP+j)*128 + p, d]
    at = nc.dram_tensor("at", [NGRP, 128, GRP * SH], FP8, kind="ExternalInput")
    # xp is pre-tiled: xp[i][p][q*F + f] = x[(i*CPP + q)*128 + p, f]
    xp = nc.dram_tensor("xp", [XP, 128, CPP * F], FP16, kind="ExternalInput")
    xtc = nc.dram_tensor("xtc", [F, SH], FP16, kind="ExternalInput")
    wt = nc.dram_tensor("wt", [F, F], FP16, kind="ExternalInput")
    bt = nc.dram_tensor("bt", [F, F], FP16, kind="ExternalInput")
    idr = nc.dram_tensor("idr", [1, SH], FP32, kind="ExternalInput")
    outT = nc.dram_tensor("outT", [F, SH], FP32, kind="ExternalOutput")

    with tile.TileContext(nc) as tc:
        with (
            tc.tile_pool(name="const", bufs=1) as constp,
            tc.tile_pool(name="apool", bufs=8) as apool,
            tc.tile_pool(name="outp", bufs=1) as outp,
            tc.tile_pool(name="psA", bufs=1, space=bass.MemorySpace.PSUM) as psA,
        ):
            # small consts first on the Scalar ring (res matmuls unblock early)
            bt_sb = constp.tile([F, F], FP16, tag="bt")
            nc.scalar.dma_start(bt_sb[:], bt[:])
            xtc_sb = constp.tile([F, SH], FP16, tag="xtc")
            nc.scalar.dma_start(xtc_sb[:], xtc[:])
            wt_sb = constp.tile([F, F], FP16, tag="wt")
            nc.scalar.dma_start(wt_sb[:], wt[:])
            xp_sb = [
                constp.tile([128, CPP * F], FP16, tag=f"xp{i}", name=f"xp{i}")
                for i in range(XP)
            ]
            for i in range(XP):
                nc.scalar.dma_start(xp_sb[i][:], xp[i])

            psy = psA.tile([128, SH], FP32, tag="y")    # Y = x^T @ A^T
            ps = psA.tile([128, SH], FP32, tag="agg")   # res + W @ Y

            # residual: ps[f, n] = sum_c B^T[c, f] * (deg*x)^T[c, n]
            for b in range(NB):
                nc.tensor.matmul(
                    ps[:, ts(b, 512)],
                    bt_sb[:],
                    xtc_sb[:, ts(b, 512)],
                    start=True,
                    stop=False,
                )

            # main loop: Y[c, dst] += sum_src x[src, c] * A^T[src, dst]
            for g in range(NGRP):
                a_t = apool.tile([128, GRP * SH], FP8, tag="a")
                if g == 0:
                    half = GRP * SH // 2
                    nc.sync.dma_start(a_t[:, 0:half], at[g][:, 0:half])
                    nc.sync.dma_start(a_t[:, half:], at[g][:, half:])
                else:
                    nc.sync.dma_start(a_t[:], at[g])
                for j in range(GRP):
                    c = g * GRP + j
                    last = c == NCH - 1
                    for b in range(NB):
                        nc.tensor.matmul(
                            psy[:, ts(b, 512)],
                            xp_sb[c // CPP][:, ts(c % CPP, 128)],
                            a_t[:, bass.ds(j * SH + b * 512, 512)],
                            start=(c == 0),
                            stop=last,
                        )

            # invdeg broadcast across partitions via stride-0 DMA replicate
            # (issued late on the Scalar ring; only needed at eviction)
            idb_sb = constp.tile([128, SH], FP32, tag="idb")
            nc.scalar.dma_start(idb_sb[:], bass.AP(idr, 0, [[0, 128], [1, SH]]))

            # tail, pipelined per bank: cast Y to fp16, apply W, scale, DMA out
            y_sb = constp.tile([128, SH], FP16, tag="ysb")
            out_sb = outp.tile([128, SH], FP32, tag="out")
            for b in range(NB):
                nc.vector.tensor_copy(y_sb[:, ts(b, 512)], psy[:, ts(b, 512)])
                nc.tensor.matmul(
                    ps[:, ts(b, 512)],
                    wt_sb[:],
                    y_sb[:, ts(b, 512)],
                    start=False,
                    stop=True,
                )
                nc.vector.tensor_mul(
                    out_sb[:, ts(b, 512)], ps[:, ts(b, 512)], idb_sb[:, ts(b, 512)]
                )
                nc.scalar.dma_start(outT[:, ts(b, 512)], out_sb[:, ts(b, 512)])

    nc.compile()
    _NC = nc
    return nc


def _prep_inputs(x, edge_index, W, B):
    src = np.asarray(edge_index[0])
    dst = np.asarray(edge_index[1])
    x = np.asarray(x, dtype=np.float32)
    W = np.asarray(W, dtype=np.float32)
    B = np.asarray(B, dtype=np.float32)

    deg = np.bincount(dst, minlength=N).astype(np.float32)
    dtil = np.where(deg == 0, np.float32(1.0), deg)
    invdeg = (np.float32(1.0) / dtil).astype(np.float32)

    # A^T shards pre-tiled to the SBUF layout the kernel consumes:
    # at_all[core, g, p, j*SH + d] = A^T[src=(g*GRP+j)*128 + p, dst_rel=d]
    # (scatter with duplicate edges naturally collapses - set semantics).
    at_all = np.zeros((NCORES, NGRP, 128, GRP * SH), dtype=np.uint8)
    g = src >> 9             # src // (GRP*128)
    p = src & 127
    j = (src >> 7) & (GRP - 1)
    at_all[dst >> SHB, g, p, (j << SHB) | (dst & (SH - 1))] = 0x38
    at_all = at_all.view(ml_dtypes.float8_e4m3)

    # x pre-tiled for stationary chunks: [XP, 128, CPP*F]
    xp_np = np.ascontiguousarray(
        x.astype(np.float16).reshape(XP, CPP, 128, F).transpose(0, 2, 1, 3)
    ).reshape(XP, 128, CPP * F)
    xtil = (dtil[:, None] * x).astype(np.float16)
    wt_np = np.ascontiguousarray(W.T).astype(np.float16)
    bt_np = np.ascontiguousarray(B.T).astype(np.float16)

    in_maps = []
    for k in range(NCORES):
        sl = slice(k * SH, (k + 1) * SH)
        in_maps.append(
            {
                "at": at_all[k],
                "xp": xp_np,
                "xtc": np.ascontiguousarray(xtil[sl].T),
                "wt": wt_np,
                "bt": bt_np,
                "idr": np.ascontiguousarray(invdeg[None, sl]),
            }
        )
    return in_maps


def kernel(x, edge_index, W, B):
    nc = _build()
    in_maps = _prep_inputs(x, edge_index, W, B)
    res = run_bass_kernel_spmd(nc, in_maps, core_ids=list(range(NCORES)))
    out = np.empty((N, F), dtype=np.float32)
    for k in range(NCORES):
        out[k * SH : (k + 1) * SH, :] = res.results[k]["outT"].T
    return out


# revision 17
# speedup vs baseline: 27313.4373x; 1.0142x over previous
"""Trainium2 Bass kernel for MatrixGraphConvolution.

out = D^-1 A (x @ W.T) + x @ B.T,  A[dst,src]=1 (set semantics),
deg counts duplicate edges, N=16384, E=524288, F=128.

Strategy (8 NeuronCores, row-sharded by dst):
  * Host builds the 0/1 adjacency-transpose shards A^T[src, dst_rel] as
    fp8e4m3 (values 0/1 are exact), laid out in the exact SBUF tile
    order so every A DMA is fully contiguous - 32MB per core, the
    dominant HBM read (streamed on the Sync HWDGE ring; everything else
    rides the Scalar ring).
  * Associativity: A@(x@W.T) = ((x^T @ A^T)^T @ W.T). The main loop
    accumulates Y[f_in, dst] = x^T @ A^T over 128 src-chunks in PSUM
    with raw x chunks as the stationary operand (no xW precompute
    phase) and the fp8 A^T as the moving operand (mixed fp16 x fp8
    matmul - verified exact on HW). W is applied once at the end:
    agg^T = W @ Y (4 matmuls).
  * Residual x @ B.T accumulates in the second PSUM region with x
    pre-scaled by deg on the host, so the final per-dst 1/deg scale
    applies to both terms: out = invdeg * (A xW + deg * (x B^T)).
  * invdeg row is partition-broadcast on-device via a stride-0 DMA,
    then one fp32 multiply per PSUM bank at eviction, DMA out^T.
"""

import sys

sys.path.insert(0, "/opt/trn_rl_repo")

import numpy as np
import ml_dtypes

import concourse.bass as bass
import concourse.tile as tile
import concourse.mybir as mybir
from concourse import bacc
from concourse.bass import ts
from concourse.bass_utils import run_bass_kernel_spmd

N, E, F = 16384, 524288, 128
NCORES = 8
SH = N // NCORES          # 2048 dst rows per core
SHB = 11                  # log2(SH)
NCH = N // 128            # 128 src chunks
GRP = 4                   # src chunks per DMA group (1MB per DMA)
NGRP = NCH // GRP
NB = SH // 512            # moving-dim blocks (PSUM banks) per chunk
XP = 8                    # x pieces (16 chunks each)
CPP = NCH // XP           # chunks per x piece

FP16 = mybir.dt.float16
FP8 = mybir.dt.float8e4
FP32 = mybir.dt.float32

_NC = None


def _build():
    global _NC
    if _NC is not None:
        return _NC
    nc = bacc.Bacc(None, target_bir_lowering=False)
    # at is pre-tiled on the host: at[g][p][j*SH + d] = A^T[(g*GRP+j)*128 + p, d]
    at = nc.dram_tensor("at", [NGRP, 128, GRP * SH], FP8, kind="ExternalInput")
    # xp is pre-tiled: xp[i][p][q*F + f] = x[(i*CPP + q)*128 + p, f]
    xp = nc.dram_tensor("xp", [XP, 128, CPP * F], FP16, kind="ExternalInput")
    xtc = nc.dram_tensor("xtc", [F, SH], FP16, kind="ExternalInput")
    wt = nc.dram_tensor("wt", [F, F], FP16, kind="ExternalInput")
    bt = nc.dram_tensor("bt", [F, F], FP16, kind="ExternalInput")
    idr = nc.dram_tensor("idr", [1, SH], FP32, kind="ExternalInput")
    outT = nc.dram_tensor("outT", [F, SH], FP32, kind="ExternalOutput")

    with tile.TileContext(nc) as tc:
        with (
            tc.tile_pool(name="const", bufs=1) as constp,
            tc.tile_pool(name="apool", bufs=8) as apool,
            tc.tile_pool(name="outp", bufs=1) as outp,
            tc.tile_pool(name="psA", bufs=1, space=bass.MemorySpace.PSUM) as psA,
        ):
            # small consts first on the Scalar ring (res matmuls unblock early)
            bt_sb = constp.tile([F, F], FP16, tag="bt")
            nc.scalar.dma_start(bt_sb[:], bt[:])
            xtc_sb = constp.tile([F, SH], FP16, tag="xtc")
            nc.scalar.dma_start(xtc_sb[:], xtc[:])
            wt_sb = constp.tile([F, F], FP16, tag="wt")
            nc.scalar.dma_start(wt_sb[:], wt[:])
            xp_sb = [
                constp.tile([128, CPP * F], FP16, tag=f"xp{i}", name=f"xp{i}")
                for i in range(XP)
            ]
            for i in range(XP):
                nc.scalar.dma_start(xp_sb[i][:], xp[i])

            psy = psA.tile([128, SH], FP32, tag="y")    # Y = x^T @ A^T
            ps = psA.tile([128, SH], FP32, tag="agg")   # res + W @ Y

            # residual: ps[f, n] = sum_c B^T[c, f] * (deg*x)^T[c, n]
            for b in range(NB):
                nc.tensor.matmul(
                    ps[:, ts(b, 512)],
                    bt_sb[:],
                    xtc_sb[:, ts(b, 512)],
                    start=True,
                    stop=False,
                )

            # main loop: Y[c, dst] += sum_src x[src, c] * A^T[src, dst]
            for g in range(NGRP):
                a_t = apool.tile([128, GRP * SH], FP8, tag="a")
                if g == 0:
                    half = GRP * SH // 2
                    nc.sync.dma_start(a_t[:, 0:half], at[g][:, 0:half])
                    nc.sync.dma_start(a_t[:, half:], at[g][:, half:])
                else:
                    nc.sync.dma_start(a_t[:], at[g])
                for j in range(GRP):
                    c = g * GRP + j
                    last = c == NCH - 1
                    for b in range(NB):
                        nc.tensor.matmul(
                            psy[:, ts(b, 512)],
                            xp_sb[c // CPP][:, ts(c % CPP, 128)],
                            a_t[:, bass.ds(j * SH + b * 512, 512)],
                            start=(c == 0),
                            stop=last,
                        )

            # invdeg broadcast: 8KB row load + on-chip GpSimd partition
            # broadcast (saves 1MB of HBM traffic; GpSimd is otherwise idle)
            idr_sb = constp.tile([1, SH], FP32, tag="idr")
            nc.scalar.dma_start(idr_sb[:], idr[:])
            idb_sb = constp.tile([128, SH], FP32, tag="idb")
            nc.gpsimd.partition_broadcast(idb_sb[:], idr_sb[:])

            # tail, pipelined per bank: cast Y to fp16, apply W, scale, DMA out
            y_sb = constp.tile([128, SH], FP16, tag="ysb")
            out_sb = outp.tile([128, SH], FP32, tag="out")
            for b in range(NB):
                nc.vector.tensor_copy(y_sb[:, ts(b, 512)], psy[:, ts(b, 512)])
                nc.tensor.matmul(
                    ps[:, ts(b, 512)],
                    wt_sb[:],
                    y_sb[:, ts(b, 512)],
                    start=False,
                    stop=True,
                )
                nc.vector.tensor_mul(
                    out_sb[:, ts(b, 512)], ps[:, ts(b, 512)], idb_sb[:, ts(b, 512)]
                )
                nc.scalar.dma_start(outT[:, ts(b, 512)], out_sb[:, ts(b, 512)])

    nc.compile()
    _NC = nc
    return nc


def _prep_inputs(x, edge_index, W, B):
    src = np.asarray(edge_index[0])
    dst = np.asarray(edge_index[1])
    x = np.asarray(x, dtype=np.float32)
    W = np.asarray(W, dtype=np.float32)
    B = np.asarray(B, dtype=np.float32)

    deg = np.bincount(dst, minlength=N).astype(np.float32)
    dtil = np.where(deg == 0, np.float32(1.0), deg)
    invdeg = (np.float32(1.0) / dtil).astype(np.float32)

    # A^T shards pre-tiled to the SBUF layout the kernel consumes:
    # at_all[core, g, p, j*SH + d] = A^T[src=(g*GRP+j)*128 + p, dst_rel=d]
    # (scatter with duplicate edges naturally collapses - set semantics).
    at_all = np.zeros((NCORES, NGRP, 128, GRP * SH), dtype=np.uint8)
    g = src >> 9             # src // (GRP*128)
    p = src & 127
    j = (src >> 7) & (GRP - 1)
    at_all[dst >> SHB, g, p, (j << SHB) | (dst & (SH - 1))] = 0x38
    at_all = at_all.view(ml_dtypes.float8_e4m3)

    # x pre-tiled for stationary chunks: [XP, 128, CPP*F]
    xp_np = np.ascontiguousarray(
        x.astype(np.float16).reshape(XP, CPP, 128, F).transpose(0, 2, 1, 3)
    ).reshape(XP, 128, CPP * F)
    xtil = (dtil[:, None] * x).astype(np.float16)
    wt_np = np.ascontiguousarray(W.T).astype(np.float16)
    bt_np = np.ascontiguousarray(B.T).astype(np.float16)

    in_maps = []
    for k in range(NCORES):
        sl = slice(k * SH, (k + 1) * SH)
        in_maps.append(
            {
                "at": at_all[k],
                "xp": xp_np,
                "xtc": np.ascontiguousarray(xtil[sl].T),
                "wt": wt_np,
                "bt": bt_np,
                "idr": np.ascontiguousarray(invdeg[None, sl]),
            }
        )
    return in_maps


def kernel(x, edge_index, W, B):
    nc = _build()
    in_maps = _prep_inputs(x, edge_index, W, B)
    res = run_bass_kernel_spmd(nc, in_maps, core_ids=list(range(NCORES)))
    out = np.empty((N, F), dtype=np.float32)
    for k in range(NCORES):
        out[k * SH : (k + 1) * SH, :] = res.results[k]["outT"].T
    return out
